# revision 4
# baseline (speedup 1.0000x reference)
"""GQA attention (B=2,T=2048,D=4096, 32Q/8KV heads, RoPE, causal) on 8 TRN2 cores.

Sharding: core c = (batch b = c//4, head-group g = c%4). Each core computes its
batch's attention for 8 query heads (global 8g..8g+8) + their 2 aligned KV heads
(global 2g..2g+2), and applies its slice of wo -> a partial [T, D] output.
Host sums the 4 head-group partials per batch. No collectives.

Device kernel (per core), bf16 matmuls / f32 accumulation & softmax:
  A) QKV projections from host-pre-transposed x^T (PE, 512-wide chunks),
     RoPE on DVE via negative-step pair-swap APs reading the PSUM chunk,
     PE-transpose Q,K into [head_dim, tok] layout; V stays [tok, head_dim].
     wqkv is staged oc-major so the first projection chain only needs 1/3
     of the weight bytes; DMAs split across gpsimd+scalar queues.
  B) Per tq block (b outer) / head: 512-wide S^T strips [tk=128, tq=512]
     (lhsT=K-tile, rhs=Q^T) so exp output P^T feeds the PV matmul with no
     P transposes. Causal: strips narrowed to the valid column range,
     triangle-tile additive mask on PSUM. Softmax denominator WITHOUT the
     per-strip ones-matmul: DVE accumulates P^T strips lane-wise into an
     f32 acc tile; one cross-partition fold per (h,b) on the (otherwise
     idle) GpSimd engine via partition_all_reduce (fallback: bf16 cast +
     ones-matmul). Normalization folded into the ot->aot copyback mul.
  C) Output projection interleaved per block: block b's wo matmuls are
     emitted between block b+1's attention strips so the PE fills exp-
     latency gaps with useful work; PSUM->SBUF staging on ScalarE;
     one 2MB out DMA per token tile.
"""
import numpy as np
import ml_dtypes

import concourse.bass as bass
import concourse.bass_isa as bass_isa
import concourse.mybir as mybir
from concourse import bacc, tile
from concourse.bass_utils import run_bass_kernel_spmd

bf16 = mybir.dt.bfloat16
f32 = mybir.dt.float32
BF = ml_dtypes.bfloat16

B, T, D = 2, 2048, 4096
NQ, NKV, HD = 32, 8, 128
HQ, HKV = 8, 2            # per-core heads
NT = T // 128             # 16 token tiles
NC = D // 128             # 32 contraction chunks
NB = NT // 4              # 4 tq blocks of 512
SCALE = 1.0 / np.sqrt(HD)
NEG = -1e9

USE_GPSIMD_ALLREDUCE = True   # fold acc across partitions on GpSimd (else PE)


def _build_nc():
    nc = bacc.Bacc(None, target_bir_lowering=False)
    xt_ext = nc.declare_dram_parameter("xt", [NT, 128, NC, 128], bf16, isOutput=False)
    wqkv_ext = nc.declare_dram_parameter("wqkv", [128, 3, NC, 512], bf16, isOutput=False)
    wo_ext = nc.declare_dram_parameter("wo", [128, HQ, D], bf16, isOutput=False)
    rope_ext = nc.declare_dram_parameter("rope", [128, NT, 1024], bf16, isOutput=False)
    mask_ext = nc.declare_dram_parameter("mask", [128, 4, 512], bf16, isOutput=False)
    id_ext = nc.declare_dram_parameter("ident", [128, 128], bf16, isOutput=False)
    out_ext = nc.declare_dram_parameter("out", [T, D], f32, isOutput=True)

    with tile.TileContext(nc) as tc:
        with (
            tc.tile_pool(name="persist", bufs=1) as persist,
        ):
            # per-tq-block tiles so phase B can start before phase A ends
            qtb = [persist.tile([128, HQ, 512], bf16, tag=f"qt{j}", name=f"qt{j}") for j in range(NB)]
            ktb = [persist.tile([128, HKV, 512], bf16, tag=f"kt{j}", name=f"kt{j}") for j in range(NB)]
            vbb = [persist.tile([128, 4 * 256], bf16, tag=f"vb{j}", name=f"vb{j}") for j in range(NB)]
            ident = persist.tile([128, 128], bf16, tag="ident")
            masks = persist.tile([128, 4, 512], bf16, tag="mask")
            scr = persist.tile([128, 8], f32, tag="scr")

            # ---------------- Phase A: projections + rope + transposes -------
            with (
                tc.tile_pool(name="wqkvp", bufs=1) as wqkvp,
                tc.tile_pool(name="xtp", bufs=2) as xtp,
                tc.tile_pool(name="ropep", bufs=2) as ropep,
                tc.tile_pool(name="rotp", bufs=2) as rotp,
                tc.tile_pool(name="psA", bufs=6, space="PSUM") as psA,
                tc.tile_pool(name="ptA", bufs=2, space="PSUM") as ptA,
            ):
                # oc-major weight staging: first projection chain needs only
                # wqkv[:, 0]; split each slice across two DMA queues.
                wqkv = wqkvp.tile([128, 3, NC, 512], bf16, tag="wqkv")
                nc.gpsimd.dma_start(wqkv[:, 0, 0:8], wqkv_ext[:, 0, 0:8])
                nc.scalar.dma_start(wqkv[:, 0, 8:16], wqkv_ext[:, 0, 8:16])
                nc.gpsimd.dma_start(wqkv[:, 0, 16:24], wqkv_ext[:, 0, 16:24])
                nc.scalar.dma_start(wqkv[:, 0, 24:32], wqkv_ext[:, 0, 24:32])
                nc.gpsimd.dma_start(ident[:], id_ext[:])
                nc.scalar.dma_start(masks[:], mask_ext[:])
                nc.gpsimd.dma_start(wqkv[:, 1, 0:16], wqkv_ext[:, 1, 0:16])
                nc.scalar.dma_start(wqkv[:, 1, 16:32], wqkv_ext[:, 1, 16:32])
                nc.gpsimd.dma_start(wqkv[:, 2, 0:16], wqkv_ext[:, 2, 0:16])
                nc.scalar.dma_start(wqkv[:, 2, 16:32], wqkv_ext[:, 2, 16:32])
                # preload the Exp activation table while ScalarE is idle
                nc.vector.memset(scr[:], 0.0)
                nc.scalar.activation(scr[:], scr[:],
                                     mybir.ActivationFunctionType.Exp,
                                     bias=0.0, scale=1.0)

                for tau in range(NT):
                    xts = xtp.tile([128, NC, 128], bf16, tag="xt")
                    nc.sync.dma_start(xts[:], xt_ext[tau])
                    rope = ropep.tile([128, 1024], bf16, tag="rope")
                    nc.sync.dma_start(rope[:], rope_ext[:, tau, :])
                    cc, ss = rope[:, 0:512], rope[:, 512:1024]

                    for oc in range(3):
                        ps = psA.tile([128, 512], f32, tag="proj")
                        for c in range(NC):
                            nc.tensor.matmul(
                                ps[:], xts[:, c, :], wqkv[:, oc, c, :],
                                start=(c == 0), stop=(c == NC - 1))
                        rt = rotp.tile([128, 1024], bf16, tag="rot")
                        rot, tmp = rt[:, 0:512], rt[:, 512:1024]
                        if oc < 2:  # 4 q heads
                            _rope(nc, ps[:], cc, ss, rot, tmp)
                            pt = ptA.tile([128, 512], bf16, tag="ptA")
                            for j in range(4):
                                nc.tensor.transpose(
                                    pt[:, j * 128:(j + 1) * 128],
                                    rot[:, j * 128:(j + 1) * 128], ident[:])
                            nc.vector.tensor_copy(
                                qtb[tau // 4][:, oc * 4:(oc + 1) * 4,
                                              (tau % 4) * 128:(tau % 4 + 1) * 128],
                                pt[:].rearrange("p (h t) -> p h t", h=4))
                        else:  # 2 k heads + 2 v heads
                            _rope(nc, ps[:, 0:256], cc[:, 0:256], ss[:, 0:256],
                                  rot[:, 0:256], tmp[:, 0:256])
                            pt = ptA.tile([128, 512], bf16, tag="ptA")
                            for j in range(2):
                                nc.tensor.transpose(
                                    pt[:, j * 128:(j + 1) * 128],
                                    rot[:, j * 128:(j + 1) * 128], ident[:])
                            nc.vector.tensor_copy(
                                ktb[tau // 4][:, :, (tau % 4) * 128:(tau % 4 + 1) * 128],
                                pt[:, 0:256].rearrange("p (h t) -> p h t", h=2))
                            nc.vector.tensor_copy(
                                vbb[tau // 4][:, (tau % 4) * 256:(tau % 4 + 1) * 256],
                                ps[:, 256:512])

            # ---------------- Phase B+C interleaved --------------------------
            with (
                tc.tile_pool(name="wop", bufs=1) as wop,
                tc.tile_pool(name="aotp", bufs=NB) as aotp,
                tc.tile_pool(name="ptsp", bufs=4) as ptsp,
                tc.tile_pool(name="accp", bufs=2) as accp,
                tc.tile_pool(name="rssb", bufs=2) as rssb,
                tc.tile_pool(name="recp", bufs=2) as recp,
                tc.tile_pool(name="outp", bufs=2) as outp,
                tc.tile_pool(name="onep", bufs=1) as onep,
                tc.tile_pool(name="psB", bufs=3, space="PSUM") as psB,
                tc.tile_pool(name="otB", bufs=2, space="PSUM") as otB,
                tc.tile_pool(name="psC", bufs=3, space="PSUM") as psC,
            ):
                wo = None        # allocated after block 0 (waits on wqkv free)
                aotb = [None] * NB
                if not USE_GPSIMD_ALLREDUCE:
                    ones = onep.tile([128, 128], bf16, tag="ones")
                    nc.vector.memset(ones[:], 1.0)

                def emit_head(b, h):
                    kvh = h // 4
                    nstrip = 4 * (b + 1)
                    acc = accp.tile([128, 512], f32, tag="acc")
                    ot = otB.tile([128, 512], f32, tag="ot")
                    for t in range(nstrip):
                        # diag strips: only columns f >= 128r are valid
                        r = t - 4 * b
                        lo = 128 * r if r > 0 else 0
                        s_ps = psB.tile([128, 512], f32, tag="s")
                        nc.tensor.matmul(
                            s_ps[:, lo:512],
                            ktb[t // 4][:, kvh, (t % 4) * 128:(t % 4 + 1) * 128],
                            qtb[b][:, h, lo:512],
                            start=True, stop=True)
                        if r >= 0:  # triangle tile only
                            nc.vector.tensor_add(
                                s_ps[:, 128 * r:128 * (r + 1)],
                                s_ps[:, 128 * r:128 * (r + 1)],
                                masks[:, r, 128 * r:128 * (r + 1)])
                        pts = ptsp.tile([128, 512], bf16, tag="pts")
                        nc.scalar.activation(
                            pts[:, lo:512], s_ps[:, lo:512],
                            mybir.ActivationFunctionType.Exp,
                            bias=0.0, scale=SCALE)
                        # lane-wise strip accumulation on DVE (replaces the
                        # per-strip ones-matmul rowsum on the PE)
                        if t == 0:
                            nc.vector.tensor_copy(acc[:], pts[:])
                        else:
                            nc.vector.tensor_add(
                                acc[:, lo:512], acc[:, lo:512], pts[:, lo:512])
                        nc.tensor.matmul(
                            ot[:, lo:512],
                            vbb[t // 4][:, (t % 4) * 256 + kvh * 128:
                                        (t % 4) * 256 + (kvh + 1) * 128],
                            pts[:, lo:512],
                            start=(t == 0), stop=(t == nstrip - 1))
                    recip = recp.tile([128, 512], f32, tag="recip")
                    if USE_GPSIMD_ALLREDUCE:
                        rs = rssb.tile([128, 512], f32, tag="rs")
                        nc.gpsimd.partition_all_reduce(
                            rs[:], acc[:], channels=128,
                            reduce_op=bass_isa.ReduceOp.add)
                        nc.vector.reciprocal_approx_fast(out=recip[:], in_=rs[:])
                    else:
                        rs_ps = psC.tile([128, 512], f32, tag="rsf")
                        acc16 = rssb.tile([128, 512], bf16, tag="rs")
                        nc.scalar.copy(acc16[:], acc[:])
                        nc.tensor.matmul(rs_ps[:], ones[:], acc16[:],
                                         start=True, stop=True)
                        nc.vector.reciprocal_approx_fast(out=recip[:], in_=rs_ps[:])
                    nc.vector.tensor_mul(
                        aotb[b][:, h, 0:512], ot[:], recip[:])

                ostages = {}

                def make_c_thunks(b):
                    def mk(tau, half):
                        def thunk():
                            if half == 0:
                                ostages[tau] = outp.tile(
                                    [128, D], f32, tag="ostage",
                                    name=f"ostage{tau}")
                            ostage = ostages[tau]
                            for nck in range(half * 4, half * 4 + 4):
                                o_ps = psC.tile([128, 512], f32, tag="o")
                                for h in range(HQ):
                                    nc.tensor.matmul(
                                        o_ps[:],
                                        aotb[b][:, h, (tau % 4) * 128:(tau % 4 + 1) * 128],
                                        wo[:, h, nck * 512:(nck + 1) * 512],
                                        start=(h == 0), stop=(h == HQ - 1))
                                nc.scalar.copy(
                                    ostage[:, nck * 512:(nck + 1) * 512], o_ps[:])
                            if half == 1:
                                nc.sync.dma_start(
                                    out_ext[tau * 128:(tau + 1) * 128, :],
                                    ostage[:, :])
                                del ostages[tau]
                        return thunk
                    return [mk(tau, half)
                            for tau in range(4 * b, 4 * b + 4)
                            for half in range(2)]

                pending = []
                for b in range(NB):
                    aotb[b] = aotp.tile([128, HQ, 512], bf16, tag="aot",
                                        name=f"aot{b}")
                    for h in range(HQ):
                        emit_head(b, h)
                        if pending:
                            pending.pop(0)()
                    if b == 0:
                        # allocate wo only now: its SBUF space is freed by
                        # wqkv at the end of phase A; allocating earlier
                        # would pile WAR-blocked instructions on the queues.
                        wo = wop.tile([128, HQ, D], bf16, tag="wo")
                        nc.gpsimd.dma_start(wo[:, 0:4, :], wo_ext[:, 0:4, :])
                        nc.scalar.dma_start(wo[:, 4:8, :], wo_ext[:, 4:8, :])
                    pending.extend(make_c_thunks(b))
                while pending:
                    pending.pop(0)()

    nc.compile()
    return nc


def _rope(nc, ps, cc, ss, rot, tmp):
    """rot = ps*cc + pairswap(ps)*ss   (pairs are consecutive elements)."""
    swap = ps.rearrange("p (i two) -> p i two", two=2)[:, :, ::-1]
    nc.vector.tensor_mul(tmp.rearrange("p (i two) -> p i two", two=2), swap,
                         ss.rearrange("p (i two) -> p i two", two=2))
    nc.vector.tensor_mul(rot, ps, cc)
    nc.vector.tensor_add(rot, rot, tmp)


_NC_CACHE = None


def _get_nc():
    global _NC_CACHE
    if _NC_CACHE is None:
        _NC_CACHE = _build_nc()
    return _NC_CACHE


def _rope_tables():
    i = np.arange(HD // 2, dtype=np.float64)
    theta = np.power(10000.0, -2.0 * i / HD)
    ang = np.outer(np.arange(T, dtype=np.float64), theta)    # [T, 64]
    cos, sin = np.cos(ang), np.sin(ang)
    cc128 = np.repeat(cos, 2, axis=1)                        # [T, 128]
    ss128 = np.stack([-sin, sin], axis=-1).reshape(T, HD)    # [T, 128]
    cc = np.tile(cc128, (1, 4))                              # [T, 512]
    ss = np.tile(ss128, (1, 4))
    ropeccss = np.concatenate([cc, ss], axis=1)              # [T, 1024]
    return np.ascontiguousarray(
        ropeccss.reshape(NT, 128, 1024).transpose(1, 0, 2)).astype(BF)


def _masks():
    # maskT for S^T strips: partition p = tk within strip, free f = tq within
    # block; strip r (0..3) inside the diagonal region. Valid iff tq >= tk.
    p = np.arange(128)[:, None, None]
    r = np.arange(4)[None, :, None]
    f = np.arange(512)[None, None, :]
    return np.where(f >= 128 * r + p, 0.0, NEG).astype(BF)


def _prep_core_inputs(x, wq, wk, wv, wo):
    rope = _rope_tables()
    masks = _masks()
    ident = np.eye(128).astype(BF)
    in_maps = []
    for c in range(8):
        b, g = c // 4, c % 4
        xb = np.asarray(x[b], dtype=np.float32)
        xt = np.ascontiguousarray(
            xb.reshape(NT, 128, NC, 128).transpose(0, 3, 2, 1)).astype(BF)
        wq_g = wq[:, g * 8 * HD:(g + 1) * 8 * HD]
        wk_g = wk[:, g * 2 * HD:(g + 1) * 2 * HD]
        wv_g = wv[:, g * 2 * HD:(g + 1) * 2 * HD]
        W = np.concatenate([wq_g, wk_g, wv_g], axis=1)       # [D, 1536]
        # oc-major staging: wqkv_t[p, oc, c, j] = W[c*128+p, oc*512+j]
        wqkv_t = np.ascontiguousarray(
            W.reshape(NC, 128, 3, 512).transpose(1, 2, 0, 3)).astype(BF)
        wo_g = wo[g * 8 * HD:(g + 1) * 8 * HD, :]            # [1024, D]
        wo_t = np.ascontiguousarray(
            wo_g.reshape(HQ, 128, D).transpose(1, 0, 2)).astype(BF)
        in_maps.append({
            "xt": xt, "wqkv": wqkv_t, "wo": wo_t,
            "rope": rope, "mask": masks, "ident": ident,
        })
    return in_maps


def _run(inputs, trace=False, trace_kwargs=None):
    x = np.asarray(inputs["x"], dtype=np.float32)
    wq = np.asarray(inputs["wq"], dtype=np.float32)
    wk = np.asarray(inputs["wk"], dtype=np.float32)
    wv = np.asarray(inputs["wv"], dtype=np.float32)
    wo = np.asarray(inputs["wo"], dtype=np.float32)
    nc = _get_nc()
    in_maps = _prep_core_inputs(x, wq, wk, wv, wo)
    res = run_bass_kernel_spmd(nc, in_maps, core_ids=list(range(8)),
                               trace=trace, **(trace_kwargs or {}))
    out = np.zeros((B, T, D), dtype=np.float32)
    for c in range(8):
        out[c // 4] += res.results[c]["out"]
    return out, res


def kernel(**inputs):
    out, _ = _run(inputs)
    return out
